# revision 1
# baseline (speedup 1.0000x reference)
"""Trainium2 Bass kernel: BertCL mean-pool + NT-Xent contrastive loss.

Contract: kernel(last_hidden_states [256,512,768] f32, input_mask [256,512] f32)
-> scalar f32 loss, numerically matching the jax reference.

Strategy (8 NeuronCores, SPMD):
  Batch axis sharded STRIDED: core c owns logical batches {c, c+8, c+16, ...}
  (local j <-> logical c + 8j), so the all-gather of locals [0,16) delivers
  logical batches 0..127 (a full half of z) while locals [16,32) still stream.

  stage 1 (memory-bound): per local batch, stream [512,768] through SBUF as a
    [128, 4*768] tile and reduce the sequence axis with ones-vector matmuls
    accumulating in PSUM -> [1,768] sums staged into one SBUF row.
  Per half (16 batches): AllGather the raw sums (the reference's division by
    the mask row-sum is a per-row positive scalar that cancels exactly in the
    L2 normalization, so it is skipped); after the gather each core
    L2-normalizes the [128,768] half (with 1/tau folded into the norm),
    transposes it via PE into zT, and accumulates the one logits block that
    is ever used, S[0:64, half] = z[0:64] @ z_half.T. The collective SENDs
    are emitted mid-loop (gpsimd stream is otherwise empty); all consume
    work is emitted after the loop so collective latency never stalls the
    in-order engine streams during stage 1. The first half's gather +
    processing hides under the second half's DMA streaming; only the second
    (small, latency-bound) collective plus a ~10us chain is exposed.
  Finish: diag-masked logsumexp over rows 0..63 (exp without max-subtraction
    is safe: logits are cosines/tau in [-2,2]), strict-upper-triangle pair
    sum, final scale -> scalar.

  Measured (paired K-differential, see perf_lab.py): ~198us steady-state,
  ~175us est. single-shot vs the 140.6us per-core HBM roofline; relative
  error vs the fp32 jax reference: 4.4e-7 on hardware.

  NOTE: fused DVE ops (tensor_tensor_reduce, scalar_tensor_tensor) pass
  CoreSim but hang/crash this hardware - only plain DVE ops are used.
"""

import sys
from contextlib import ExitStack

import numpy as np

_REPO = "/opt/trn_rl_repo"
if _REPO not in sys.path:
    sys.path.insert(0, _REPO)

import concourse.bass as bass  # noqa: E402  (kept for callers/debugging)
import concourse.tile as tile  # noqa: E402
from concourse import bacc, bass_utils, mybir  # noqa: E402

N_CORES = 8
B, S, H = 256, 512, 768
B_SH = B // N_CORES  # 32 local batches per core
HALF = B_SH // 2  # 16
N_PAIR = B // 4  # 64
TAU = 0.5
F32 = mybir.dt.float32
AX = mybir.AxisListType
AF = mybir.ActivationFunctionType
NEG = -30000.0  # diagonal mask value; exp(NEG + logit) == 0 exactly in fp32


def _body(
    tc,
    x,
    ident,
    dmask,
    triu,
    cnt,
    out,
    use_collective=True,
    stages=("s1", "cc", "s2"),
):
    nc = tc.nc

    with ExitStack() as ctx:
        const = ctx.enter_context(tc.tile_pool(name="const", bufs=1))
        ones_col = const.tile([128, 1], F32)
        nc.vector.memset(ones_col[:], 1.0)
        idt = const.tile([128, 128], F32)
        nc.sync.dma_start(idt[:], ident[:])

        dram = ctx.enter_context(tc.tile_pool(name="dram", bufs=1, space="DRAM"))
        cc_in = dram.tile([B_SH, H], F32)
        shared = "Shared" if use_collective else "Local"
        # asymmetric split: gather locals [0,24) early (hides under the last 8
        # batches' streaming), locals [24,32) at the end (only 64 logical rows
        # of consume work left after the final latency-bound collective)
        SEG = [(0, 16), (16, 32)]
        cc_o = [
            dram.tile([8 * (j1 - j0), H], F32, addr_space=shared, name=f"cc_o{h}")
            for h, (j0, j1) in enumerate(SEG)
        ]

        # staging row for pooled sums: [1, 32*768] on partition 0
        pooled_sb = const.tile([1, B_SH * H], F32)

        xin = ctx.enter_context(tc.tile_pool(name="xin", bufs=6))
        ps1 = ctx.enter_context(tc.tile_pool(name="ps1", bufs=2, space="PSUM"))
        s2 = ctx.enter_context(tc.tile_pool(name="s2", bufs=1))
        s2t = ctx.enter_context(tc.tile_pool(name="s2t", bufs=2))
        psT = ctx.enter_context(tc.tile_pool(name="psT", bufs=2, space="PSUM"))
        psS = ctx.enter_context(tc.tile_pool(name="psS", bufs=1, space="PSUM"))

        # zT[:, k*256 + p] = z[p, k*128 + q] for partition q (h on partitions)
        zT = s2.tile([128, 6 * B], F32)
        pS = psS.tile([N_PAIR, B], F32)

        def send_half(h):
            """Gather raw sums for local rows [16h,16h+16).

            The reference divides pooled sums by the mask row-sum before
            L2-normalizing; that per-row positive scalar cancels exactly in
            the normalization, so we gather raw sums and normalize the
            gathered rows (same result to ~1ulp, and the pre-collective
            tail shrinks to a single DMA)."""
            j0, j1 = SEG[h]
            nc.sync.dma_start(
                cc_in[j0:j1, :],
                pooled_sb[0:1, j0 * H : j1 * H].rearrange("o (b e) -> o b e", e=H),
            )

            if use_collective:
                nc.gpsimd.collective_compute(
                    "AllGather",
                    mybir.AluOpType.bypass,
                    replica_groups=[list(range(N_CORES))],
                    ins=[cc_in[j0:j1, :].opt()],
                    outs=[cc_o[h].opt()],
                )
            else:
                n = j1 - j0
                for c in range(N_CORES):
                    nc.sync.dma_start(
                        cc_o[h][c * n : (c + 1) * n, :], cc_in[j0:j1, :]
                    )

        def consume_block(h, ja, jb, name):
            """Normalize logical rows [8*ja, 8*jb) from gather h; fill zT cols.

            Gathered row (c, j - SEG[h][0]) holds logical batch c + 8j; the
            permuted 3-D AP (j, c, e) lands partitions in logical order."""
            P = 8 * (jb - ja)  # rows in this block
            col = 8 * ja  # zT column base = first logical row
            zh = s2.tile([P, H], F32, tag=name, name=name)
            src = cc_o[h].rearrange("(c j) e -> j c e", c=N_CORES)
            nc.sync.dma_start(zh[:], src[ja - SEG[h][0] : jb - SEG[h][0]])
            sqs = s2t.tile([P, H], F32, tag=f"sqs{name}", name=f"sqs{name}")
            ssn = s2t.tile([P, 1], F32, tag=f"ssn{name}", name=f"ssn{name}")
            nc.vector.tensor_mul(sqs[:], zh[:], zh[:])
            nc.vector.reduce_sum(out=ssn[:], in_=sqs[:], axis=AX.X)
            # sqrt(TAU * ss): scales z by 1/sqrt(tau) so S = z'z'^T = logits
            nrm = s2t.tile([P, 1], F32, tag=f"nrm{name}", name=f"nrm{name}")
            nc.scalar.activation(nrm[:], ssn[:], AF.Sqrt, scale=TAU)
            rn = s2t.tile([P, 1], F32, tag=f"rn{name}", name=f"rn{name}")
            nc.vector.reciprocal(rn[:], nrm[:])
            nc.vector.tensor_scalar_mul(zh[:], zh[:], rn[:, 0:1])
            for k in range(6):
                pt = psT.tile([128, 128], F32, tag="pt")
                nc.tensor.transpose(
                    pt[:, 0:P], zh[:, k * 128 : (k + 1) * 128], idt[0:P, 0:P]
                )
                nc.vector.tensor_copy(
                    zT[:, k * B + col : k * B + col + P], pt[:, 0:P]
                )

        def logits_block(col, n):
            """S[0:64, col:col+n] += sum_k zT_k[:, 0:64].T @ zT_k[:, col:col+n]"""
            for k in range(6):
                nc.tensor.matmul(
                    pS[:, col : col + n],
                    lhsT=zT[:, k * B : k * B + N_PAIR],
                    rhs=zT[:, k * B + col : k * B + col + n],
                    start=(k == 0),
                    stop=(k == 5),
                )

        # ---- stage 1: per-batch sum over the sequence axis -------------------
        x4 = x.rearrange("b (c p) e -> b p c e", p=128)  # [32, 128, 4, 768]
        for b in range(B_SH):
            if "s1" in stages:
                xt = xin.tile([128, 4 * H], F32)
                nc.sync.dma_start(xt[:], x4[b])
                ps = ps1.tile([1, H], F32)
                for c in range(4):
                    nc.tensor.matmul(
                        ps[:, 0:512],
                        lhsT=ones_col[:, 0:1],
                        rhs=xt[:, c * H : c * H + 512],
                        start=(c == 0),
                        stop=(c == 3),
                    )
                for c in range(4):
                    nc.tensor.matmul(
                        ps[:, 512:H],
                        lhsT=ones_col[:, 0:1],
                        rhs=xt[:, c * H + 512 : (c + 1) * H],
                        start=(c == 0),
                        stop=(c == 3),
                    )
                nc.scalar.copy(pooled_sb[0:1, b * H : (b + 1) * H], ps[:])
            if "cc" in stages:
                if b == SEG[0][1] - 1:
                    send_half(0)
                elif b == SEG[1][1] - 1:
                    send_half(1)

        if "cc" not in stages or "s2" not in stages:
            return
        # each gather carries a full 128-row half of z
        consume_block(0, 0, 16, "zb0")
        logits_block(0, 128)
        consume_block(1, 16, 32, "zb1")
        logits_block(128, 128)

        # ---- finish: masked logsumexp + pair sum ----------------------------
        # pS already holds logits (1/tau folded into the normalization)
        dm = s2.tile([N_PAIR, B], F32)
        nc.sync.dma_start(dm[:], dmask[:])
        sd = s2.tile([N_PAIR, B], F32)
        nc.vector.tensor_add(sd[:], pS[:], dm[:])

        # logits are cosine/tau in [-2,2]: exp is safe without max-subtraction
        et = s2.tile([N_PAIR, B], F32)
        se = s2.tile([N_PAIR, 1], F32)
        nc.scalar.activation(et[:], sd[:], AF.Exp, scale=1.0, accum_out=se[:])
        ld = s2.tile([N_PAIR, 1], F32)
        nc.scalar.activation(ld[:], se[:], AF.Ln)  # logden

        # sum_{i<j<n} (logden[i] - logits[i,j])
        #   = sum_i cnt[i]*logden[i] - sum_ij triu[i,j]*logits[i,j]
        tri_t = s2.tile([N_PAIR, N_PAIR], F32)
        nc.sync.dma_start(tri_t[:], triu[:])
        cnt_t = s2.tile([N_PAIR, 1], F32)
        nc.sync.dma_start(cnt_t[:], cnt[:])
        mt2 = s2.tile([N_PAIR, N_PAIR], F32)
        nc.vector.tensor_mul(mt2[:], sd[0:N_PAIR, 0:N_PAIR], tri_t[:])
        rs = s2.tile([N_PAIR, 1], F32)
        nc.vector.reduce_sum(out=rs[:], in_=mt2[:], axis=AX.X)
        t1 = s2.tile([N_PAIR, 1], F32)
        nc.vector.tensor_mul(t1[:], ld[:], cnt_t[:])
        pr = s2.tile([N_PAIR, 1], F32)
        nc.vector.tensor_sub(pr[:], t1[:], rs[:])

        ptot = psS.tile([1, 1], F32, tag="ptot")
        nc.tensor.matmul(
            ptot[:], lhsT=pr[:], rhs=ones_col[0:N_PAIR, 0:1], start=True, stop=True
        )
        res = s2.tile([1, 1], F32)
        nc.vector.tensor_scalar_mul(res[:], ptot[:], -2.0 / N_PAIR * (N_PAIR - 1))
        nc.sync.dma_start(out[0:1, 0:1], res[:])


def build_nc():
    nc = bacc.Bacc("TRN2", target_bir_lowering=False, debug=False, num_devices=N_CORES)
    x = nc.dram_tensor("x", [B_SH, S, H], F32, kind="ExternalInput")
    ident = nc.dram_tensor("ident", [128, 128], F32, kind="ExternalInput")
    dmask = nc.dram_tensor("dmask", [N_PAIR, B], F32, kind="ExternalInput")
    triu = nc.dram_tensor("triu", [N_PAIR, N_PAIR], F32, kind="ExternalInput")
    cnt = nc.dram_tensor("cnt", [N_PAIR, 1], F32, kind="ExternalInput")
    out = nc.dram_tensor("loss", [1, 1], F32, kind="ExternalOutput")
    with tile.TileContext(nc) as tc:
        _body(
            tc,
            x.ap(),
            ident.ap(),
            dmask.ap(),
            triu.ap(),
            cnt.ap(),
            out.ap(),
        )
    nc.compile()
    return nc


def const_inputs():
    ident = np.eye(128, dtype=np.float32)
    dmask = np.zeros((N_PAIR, B), dtype=np.float32)
    dmask[np.arange(N_PAIR), np.arange(N_PAIR)] = NEG
    triu = np.triu(np.ones((N_PAIR, N_PAIR), dtype=np.float32), k=1)
    cnt = (N_PAIR - 1 - np.arange(N_PAIR, dtype=np.float32)).reshape(N_PAIR, 1)
    return {"ident": ident, "dmask": dmask, "triu": triu, "cnt": cnt}


def make_in_maps(last_hidden_states, input_mask):
    del input_mask  # cancels exactly in the L2 normalization (see half_tail)
    x = np.asarray(last_hidden_states, dtype=np.float32)
    consts = const_inputs()
    return [
        {"x": np.ascontiguousarray(x[c::N_CORES]), **consts}  # logical c+8j
        for c in range(N_CORES)
    ]


_CACHE = {}


def get_nc():
    if "nc" not in _CACHE:
        _CACHE["nc"] = build_nc()
    return _CACHE["nc"]


def kernel(last_hidden_states, input_mask):
    nc = get_nc()
    in_maps = make_in_maps(last_hidden_states, input_mask)
    res = bass_utils.run_bass_kernel_spmd(nc, in_maps, core_ids=list(range(N_CORES)))
    return np.asarray(res.results[0]["loss"], dtype=np.float32).reshape(())



# revision 15
# speedup vs baseline: 1.2390x; 1.2390x over previous
"""Trainium2 Bass kernel: BertCL mean-pool + NT-Xent contrastive loss (v2).

Contract: kernel(last_hidden_states [256,512,768] f32, input_mask [256,512] f32)
-> scalar f32 loss, numerically matching the jax reference.

Strategy (8 NeuronCores, SPMD), v2 = distributed logits:
  Batch axis sharded STRIDED: core c owns logical batches {c, c+8, ...}
  (local j <-> logical c + 8j).

  stage 1 (memory-bound): per local batch, stream [512,768] through SBUF as a
    [128, 4*768] tile and reduce the sequence axis with ones-vector matmuls
    accumulating in PSUM -> [1,768] raw sums staged into one SBUF row.
    (The reference's division by the mask row-sum is a positive per-row scalar
    that cancels exactly in the L2 normalization, so raw sums suffice.)

  Distributed loss: only logical rows 0..63 (= every core's locals j<8) are
    ever *anchor* rows.  After local batch 7, AllGather those 64 raw rows
    (24KB, fully hidden under streaming); every core normalizes them into
    z64T [e x 64].  Each core normalizes its OWN 32 rows (via a local DRAM
    round-trip per 8-row group) into zownT columns and accumulates the one
    S block it owns: S[0:64, own] = z64 @ z_own^T.  exp+row-accumulate give
    the per-core partial denominator Esum_own[64]; the strict-upper-triangle
    pair term uses only the group-0 columns with a per-core data mask.
    The FINAL collective is an AllGather of a [2,64] payload (512B):
    (Esum_own | pair-row-sums).  Each core sums ranks, logden = ln(Esum-e^2)
    (diag exp(S_ii)=e^{1/tau} subtracted exactly), then
    loss = -2/n*(n-1) * (sum_i cnt_i*logden_i - pair_total).

  Engine discipline: the SP(sync) ring carries ONLY the 32 x-stream DMAs so
    it never head-blocks on a collective semaphore; DRAM writes + pb load on
    the Act(scalar) HWDGE ring; consume loads + collectives on gpsimd.

  NOTE: fused DVE ops (tensor_tensor_reduce, scalar_tensor_tensor) pass
  CoreSim but hang/crash this hardware - only plain DVE ops are used.
"""

import sys
from contextlib import ExitStack

import numpy as np

_REPO = "/opt/trn_rl_repo"
if _REPO not in sys.path:
    sys.path.insert(0, _REPO)

import concourse.bass as bass  # noqa: E402  (kept for callers/debugging)
import concourse.tile as tile  # noqa: E402
from concourse import bacc, bass_utils, mybir  # noqa: E402

N_CORES = 8
B, S, H = 256, 512, 768
B_SH = B // N_CORES  # 32 local batches per core
N_PAIR = B // 4  # 64 anchor rows
G = 4  # own-row groups of 8
TAU = 0.5
E2 = float(np.exp(np.float64(1.0 / TAU)))  # exp(S_ii) subtracted from denom
F32 = mybir.dt.float32
F32R = mybir.dt.float32r
AX = mybir.AxisListType
AF = mybir.ActivationFunctionType


def _body(tc, x, ident, mask64, cnt, out, use_collective=True,
          stages=("s1", "cc", "s2")):
    nc = tc.nc
    do_s1 = "s1" in stages
    do_cc = "cc" in stages
    do_s2 = "s2" in stages and do_cc

    with ExitStack() as ctx:
        const = ctx.enter_context(tc.tile_pool(name="const", bufs=1))
        ones_col = const.tile([128, 1], F32)
        nc.vector.memset(ones_col[:], 1.0)
        ones_r = const.tile([128, 1], F32R)
        nc.vector.tensor_copy(ones_r[:], ones_col[:])  # memset can't write f32r
        idt = const.tile([128, 128], F32)
        nc.scalar.dma_start(idt[:], ident[:])
        m64 = const.tile([N_PAIR, 8], F32)
        nc.scalar.dma_start(m64[:], mask64[:])
        cnt_t = const.tile([N_PAIR, 1], F32)
        nc.scalar.dma_start(cnt_t[:], cnt[:])
        pooled_sb = const.tile([1, B_SH * H], F32)

        dram = ctx.enter_context(tc.tile_pool(name="dram", bufs=1, space="DRAM"))
        shared = "Shared" if use_collective else "Local"
        cc_in = dram.tile([8, H], F32)
        cc_o = dram.tile([64, H], F32, addr_space=shared, name="cc_o")
        rt = [dram.tile([8, H], F32, name=f"rt{g}") for g in range(1, G)]
        cc2_in = dram.tile([N_PAIR, 2], F32)
        cc2_o = dram.tile([N_CORES * N_PAIR, 2], F32, addr_space=shared,
                          name="cc2_o")

        xin = ctx.enter_context(tc.tile_pool(name="xin", bufs=6))
        ps1 = ctx.enter_context(tc.tile_pool(name="ps1", bufs=2, space="PSUM"))
        s2 = ctx.enter_context(tc.tile_pool(name="s2", bufs=1))
        s2t = ctx.enter_context(tc.tile_pool(name="s2t", bufs=2))
        psT = ctx.enter_context(tc.tile_pool(name="psT", bufs=2, space="PSUM"))
        psS = ctx.enter_context(tc.tile_pool(name="psS", bufs=1, space="PSUM"))

        # zT64[:, k*64 + l] = z64[l, k*128 + q] at partition q
        zT64 = s2.tile([128, 6 * N_PAIR], F32)
        zownT = s2.tile([128, 6 * B_SH], F32)
        pS = psS.tile([N_PAIR, B_SH], F32)
        esum = s2.tile([N_PAIR, G], F32)
        pay = s2.tile([N_PAIR, 2], F32)

        def normalize(z, P, tag):
            """L2-normalize rows of z [P, H] in place, folding 1/sqrt(tau)."""
            sqs = s2t.tile([P, H], F32, tag=f"sqs{tag}", name=f"sqs{tag}")
            ssn = s2t.tile([P, 1], F32, tag=f"ssn{tag}", name=f"ssn{tag}")
            nc.vector.tensor_mul(sqs[:], z[:], z[:])
            nc.vector.reduce_sum(out=ssn[:], in_=sqs[:], axis=AX.X)
            nrm = s2t.tile([P, 1], F32, tag=f"nrm{tag}", name=f"nrm{tag}")
            nc.scalar.activation(nrm[:], ssn[:], AF.Sqrt, scale=TAU)
            rn = s2t.tile([P, 1], F32, tag=f"rn{tag}", name=f"rn{tag}")
            nc.vector.reciprocal(rn[:], nrm[:])
            nc.vector.tensor_scalar_mul(z[:], z[:], rn[:, 0:1])

        def transpose_into(dst, dcol, z, P):
            """dst[:, dcol + k*dstride ...] <- z[P,H] transposed, 6 chunks."""
            for k in range(6):
                pt = psT.tile([128, 128], F32, tag="pt")
                nc.tensor.transpose(
                    pt[:, 0:P], z[:, k * 128 : (k + 1) * 128], idt[0:P, 0:P]
                )
                nc.vector.tensor_copy(
                    dst[:, k * dcol[1] + dcol[0] : k * dcol[1] + dcol[0] + P],
                    pt[:, 0:P],
                )

        def consume_z64():
            zh = s2.tile([N_PAIR, H], F32)
            nc.gpsimd.dma_start(
                zh[:], cc_o.rearrange("(c j) e -> j c e", c=N_CORES)
            )
            normalize(zh, N_PAIR, "z64")
            transpose_into(zT64, (0, N_PAIR), zh, N_PAIR)

        def consume_group(g):
            """Normalize own rows [8g, 8g+8) into zownT columns."""
            src = cc_in if g == 0 else rt[g - 1]
            zg = s2t.tile([8, H], F32, tag="zg", name=f"zg{g}")
            nc.gpsimd.dma_start(zg[:], src[:])
            normalize(zg, 8, "g")
            transpose_into(zownT, (8 * g, B_SH), zg, 8)

        def s_block(g):
            """pS[:, 8g:8g+8] = sum_k zT64_k.T @ zownT_k[:, 8g:8g+8]; exp."""
            for k in range(6):
                nc.tensor.matmul(
                    pS[:, 8 * g : 8 * g + 8],
                    lhsT=zT64[:, k * N_PAIR : (k + 1) * N_PAIR],
                    rhs=zownT[:, k * B_SH + 8 * g : k * B_SH + 8 * g + 8],
                    start=(k == 0),
                    stop=(k == 5),
                )
            eg = s2t.tile([N_PAIR, 8], F32, tag="eg", name=f"eg{g}")
            nc.scalar.activation(eg[:], pS[:, 8 * g : 8 * g + 8], AF.Exp,
                                 scale=1.0, accum_out=esum[:, g : g + 1])
            if g == 0:
                pm = s2t.tile([N_PAIR, 8], F32, tag="pm", name="pm")
                nc.vector.tensor_mul(pm[:], pS[:, 0:8], m64[:])
                nc.vector.reduce_sum(out=pay[:, 1:2], in_=pm[:], axis=AX.X)

        def send_group(g):
            """Stage group-g raw sums to DRAM; g==0 also AllGathers them."""
            dst = cc_in if g == 0 else rt[g - 1]
            nc.scalar.dma_start(
                dst[:],
                pooled_sb[0:1, g * 8 * H : (g + 1) * 8 * H].rearrange(
                    "o (b e) -> o b e", e=H
                ),
            )
            if g == 0:
                if use_collective:
                    nc.gpsimd.collective_compute(
                        "AllGather",
                        mybir.AluOpType.bypass,
                        replica_groups=[list(range(N_CORES))],
                        ins=[cc_in[:].opt()],
                        outs=[cc_o[:].opt()],
                    )
                else:
                    for c in range(N_CORES):
                        nc.scalar.dma_start(cc_o[c * 8 : (c + 1) * 8, :],
                                            cc_in[:])

        # ---- stage 1 + interleaved consume ----------------------------------
        x4 = x.rearrange("b (c p) e -> b p c e", p=128)  # [32, 128, 4, 768]
        for b in range(B_SH):
            if do_s1:
                # float32r: 1 PE cycle/row (fp32 is 4); same 4-byte bits
                xt = xin.tile([128, 4 * H], F32R)
                nc.sync.dma_start(xt[:], x4[b].bitcast(F32R))
                ps = ps1.tile([1, H], F32)
                for c in range(4):
                    nc.tensor.matmul(
                        ps[:, 0:512],
                        lhsT=ones_r[:, 0:1],
                        rhs=xt[:, c * H : c * H + 512],
                        start=(c == 0),
                        stop=(c == 3),
                    )
                for c in range(4):
                    nc.tensor.matmul(
                        ps[:, 512:H],
                        lhsT=ones_r[:, 0:1],
                        rhs=xt[:, c * H + 512 : (c + 1) * H],
                        start=(c == 0),
                        stop=(c == 3),
                    )
                nc.scalar.copy(pooled_sb[0:1, b * H : (b + 1) * H], ps[:])
            if do_cc:
                if b % 8 == 7:
                    send_group(b // 8)
                if do_s2:
                    if b == 15:
                        consume_group(0)
                        consume_z64()
                        s_block(0)
                    elif b == 23:
                        consume_group(1)
                        s_block(1)
                        consume_group(2)
                        s_block(2)
                    elif b == 31:
                        consume_group(3)
                        s_block(3)

        if not do_s2:
            return

        # ---- finish: payload AllGather + logden + pair sum ------------------
        nc.vector.reduce_sum(out=pay[:, 0:1], in_=esum[:], axis=AX.X)
        nc.scalar.dma_start(cc2_in[:], pay[:])
        if use_collective:
            nc.gpsimd.collective_compute(
                "AllGather",
                mybir.AluOpType.bypass,
                replica_groups=[list(range(N_CORES))],
                ins=[cc2_in[:].opt()],
                outs=[cc2_o[:].opt()],
            )
        else:
            for c in range(N_CORES):
                nc.scalar.dma_start(
                    cc2_o[c * N_PAIR : (c + 1) * N_PAIR, :], cc2_in[:]
                )

        # pb[i, 2c+v] = payload v of rank c, anchor i
        pb = s2.tile([N_PAIR, 2 * N_CORES], F32)
        nc.scalar.dma_start(
            pb[:], cc2_o.rearrange("(c i) v -> i c v", c=N_CORES)
        )
        acc = s2.tile([N_PAIR, 2], F32)
        nc.vector.tensor_add(acc[:], pb[:, 0:2], pb[:, 2:4])
        for c in range(2, N_CORES):
            nc.vector.tensor_add(acc[:], acc[:], pb[:, 2 * c : 2 * c + 2])
        # acc col 0 = Esum_total, col 1 = pair row totals
        den = s2.tile([N_PAIR, 1], F32)
        nc.vector.tensor_scalar_add(den[:], acc[:, 0:1], -E2)
        ld = s2.tile([N_PAIR, 1], F32)
        nc.scalar.activation(ld[:], den[:], AF.Ln)
        t1 = s2.tile([N_PAIR, 1], F32)
        nc.vector.tensor_mul(t1[:], ld[:], cnt_t[:])
        pr = s2.tile([N_PAIR, 1], F32)
        nc.vector.tensor_sub(pr[:], t1[:], acc[:, 1:2])
        ptot = psS.tile([1, 1], F32, tag="ptot")
        nc.tensor.matmul(
            ptot[:], lhsT=pr[:], rhs=ones_col[0:N_PAIR, 0:1], start=True,
            stop=True,
        )
        res = s2.tile([1, 1], F32)
        nc.vector.tensor_scalar_mul(res[:], ptot[:], -2.0 / N_PAIR * (N_PAIR - 1))
        nc.scalar.dma_start(out[0:1, 0:1], res[:])


def build_nc(reps=1, stages=("s1", "cc", "s2"), use_collective=True):
    nc = bacc.Bacc("TRN2", target_bir_lowering=False, debug=False,
                   num_devices=N_CORES)
    x = nc.dram_tensor("x", [B_SH, S, H], F32, kind="ExternalInput")
    ident = nc.dram_tensor("ident", [128, 128], F32, kind="ExternalInput")
    mask64 = nc.dram_tensor("mask64", [N_PAIR, 8], F32, kind="ExternalInput")
    cnt = nc.dram_tensor("cnt", [N_PAIR, 1], F32, kind="ExternalInput")
    out = nc.dram_tensor("loss", [1, 1], F32, kind="ExternalOutput")
    with tile.TileContext(nc) as tc:
        for _ in range(reps):
            _body(tc, x.ap(), ident.ap(), mask64.ap(), cnt.ap(), out.ap(),
                  use_collective=use_collective, stages=stages)
    nc.compile()
    return nc


def const_inputs(core_id):
    ident = np.eye(128, dtype=np.float32)
    i = np.arange(N_PAIR)[:, None]
    l = (core_id + 8 * np.arange(8))[None, :]
    mask64 = (i < l).astype(np.float32)  # strict upper triangle, own columns
    cnt = (N_PAIR - 1 - np.arange(N_PAIR, dtype=np.float32)).reshape(N_PAIR, 1)
    return {"ident": ident, "mask64": mask64, "cnt": cnt}


def make_in_maps(last_hidden_states, input_mask):
    del input_mask  # cancels exactly in the L2 normalization
    x = np.asarray(last_hidden_states, dtype=np.float32)
    return [
        {"x": np.ascontiguousarray(x[c::N_CORES]), **const_inputs(c)}
        for c in range(N_CORES)
    ]


_CACHE = {}


def get_nc():
    if "nc" not in _CACHE:
        _CACHE["nc"] = build_nc()
    return _CACHE["nc"]


def kernel(last_hidden_states, input_mask):
    nc = get_nc()
    in_maps = make_in_maps(last_hidden_states, input_mask)
    res = bass_utils.run_bass_kernel_spmd(nc, in_maps, core_ids=list(range(N_CORES)))
    return np.asarray(res.results[0]["loss"], dtype=np.float32).reshape(())


# revision 23
# speedup vs baseline: 1.3634x; 1.1004x over previous
"""Trainium2 Bass kernel: BertCL mean-pool + NT-Xent contrastive loss (v2).

Contract: kernel(last_hidden_states [256,512,768] f32, input_mask [256,512] f32)
-> scalar f32 loss, numerically matching the jax reference.

Strategy (8 NeuronCores, SPMD), v2 = distributed logits:
  Batch axis sharded STRIDED: core c owns logical batches {c, c+8, ...}
  (local j <-> logical c + 8j).

  stage 1 (memory-bound): per local batch, stream [512,768] through SBUF as a
    [128, 4*768] tile and reduce the sequence axis with ones-vector matmuls
    (float32r: 1 PE cycle/row vs fp32's 4) accumulating in PSUM -> [1,768]
    raw sums staged into one SBUF row.  (The reference's division by the mask
    row-sum is a positive per-row scalar that cancels exactly in the L2
    normalization, so raw sums suffice.)

  Distributed loss: only logical rows 0..63 (= every core's locals j<8) are
    ever *anchor* rows.  After local batch 7, AllGather those 64 raw rows
    (24KB, fully hidden under streaming); every core normalizes them into
    z64T [e x 64].  Each core normalizes its OWN 32 rows (via a local DRAM
    round-trip per 8-row group) into zownT columns and accumulates the one
    S block it owns: S[0:64, own] = z64 @ z_own^T.  exp+row-accumulate give
    the per-core partial denominator Esum_own[64]; the strict-upper-triangle
    pair term uses only the group-0 columns with a per-core data mask.
    The FINAL collective is an AllGather of a [64,2] payload (512B):
    (Esum_own | pair-row-sums).  Rank-summing uses two selector matmuls,
    logden = ln(Esum-e^2) (diag exp(S_ii)=e^{1/tau} subtracted exactly), then
    loss = -2/n*(n-1) * (sum_i cnt_i*logden_i - pair_total).

  Engine discipline: the SP(sync) ring carries ONLY the 32 x-stream DMAs so
    it never head-blocks on a collective semaphore; DRAM stage-writes on the
    Act(scalar) HWDGE ring; consume loads, collectives, and all post-AG2
    DMAs on gpsimd so the Act ring frees up for the next rep's PSUM drains.

  NOTE: fused DVE ops (tensor_tensor_reduce, scalar_tensor_tensor) pass
  CoreSim but hang/crash this hardware - only plain DVE ops are used.
"""

import sys
from contextlib import ExitStack

import numpy as np

_REPO = "/opt/trn_rl_repo"
if _REPO not in sys.path:
    sys.path.insert(0, _REPO)

import concourse.bass as bass  # noqa: E402  (kept for callers/debugging)
import concourse.tile as tile  # noqa: E402
from concourse import bacc, bass_utils, mybir  # noqa: E402

N_CORES = 8
B, S, H = 256, 512, 768
B_SH = B // N_CORES  # 32 local batches per core
N_PAIR = B // 4  # 64 anchor rows
G = 4  # own-row groups of 8
TAU = 0.5
E2 = float(np.exp(np.float64(1.0 / TAU)))  # exp(S_ii) subtracted from denom
F32 = mybir.dt.float32
F32R = mybir.dt.float32r
AX = mybir.AxisListType
AF = mybir.ActivationFunctionType


def _consts(tc, ctx, ident, mask64, cnt, sel):
    nc = tc.nc
    const = ctx.enter_context(tc.tile_pool(name="const", bufs=1))
    c = {}
    c["ones"] = const.tile([128, 1], F32, name="ones")
    nc.vector.memset(c["ones"][:], 1.0)
    c["ones_r"] = const.tile([128, 1], F32R, name="ones_r")
    nc.vector.tensor_copy(c["ones_r"][:], c["ones"][:])  # memset can't do f32r
    c["idt"] = const.tile([128, 128], F32, name="idt")
    nc.scalar.dma_start(c["idt"][:], ident[:])
    c["m64"] = const.tile([N_PAIR, 8], F32, name="m64")
    nc.scalar.dma_start(c["m64"][:], mask64[:])
    c["cntR"] = const.tile([1, N_PAIR], F32, name="cntR")  # cnt as a row
    nc.scalar.dma_start(c["cntR"][:], cnt.rearrange("i o -> o i"))
    c["sel"] = const.tile([2 * N_CORES, 2], F32, name="selt")
    nc.scalar.dma_start(c["sel"][:], sel[:])
    return c


def _body(tc, C, x, out, use_collective=True, stages=("s1", "cc", "s2")):
    nc = tc.nc
    do_s1 = "s1" in stages
    do_cc = "cc" in stages
    do_s2 = "s2" in stages and do_cc
    ones, ones_r, idt, m64 = C["ones"], C["ones_r"], C["idt"], C["m64"]

    with ExitStack() as ctx:
        # Shared collective outputs must have exactly one writer, so the
        # DRAM tiles are per-rep
        dram = ctx.enter_context(tc.tile_pool(name="dram", bufs=1, space="DRAM"))
        cc_in = dram.tile([8, H], F32, name="cc_in")
        cc_o = dram.tile([64, H], F32, addr_space="Shared", name="cc_o")
        rt = [dram.tile([8, H], F32, name=f"rt{g}") for g in range(1, G)]
        cc2_in = dram.tile([2, N_PAIR], F32, name="cc2_in")
        cc2_o = dram.tile([2 * N_CORES, N_PAIR], F32, addr_space="Shared",
                          name="cc2_o")

        work = ctx.enter_context(tc.tile_pool(name="work", bufs=1))
        pooled_sb = work.tile([1, B_SH * H], F32)

        xin = ctx.enter_context(tc.tile_pool(name="xin", bufs=6))
        ps1 = ctx.enter_context(tc.tile_pool(name="ps1", bufs=2, space="PSUM"))
        s2 = ctx.enter_context(tc.tile_pool(name="s2", bufs=1))
        s2t = ctx.enter_context(tc.tile_pool(name="s2t", bufs=2))
        psT = ctx.enter_context(tc.tile_pool(name="psT", bufs=1, space="PSUM"))
        psS = ctx.enter_context(tc.tile_pool(name="psS", bufs=1, space="PSUM"))

        # zT64[:, k*64 + l] = z64[l, k*128 + q] at partition q
        zT64 = s2.tile([128, 6 * N_PAIR], F32)
        zownT = s2.tile([128, 6 * B_SH], F32)
        pS = psS.tile([N_PAIR, B_SH], F32)
        esum = s2.tile([N_PAIR, G], F32)
        pay = s2.tile([N_PAIR, 2], F32)

        def normalize(z, P, tag):
            """L2-normalize rows of z [P, H] in place, folding 1/sqrt(tau)."""
            sqs = s2t.tile([P, H], F32, tag=f"sqs{tag}", name=f"sqs{tag}")
            ssn = s2t.tile([P, 1], F32, tag=f"ssn{tag}", name=f"ssn{tag}")
            nc.vector.tensor_mul(sqs[:], z[:], z[:])
            nc.vector.reduce_sum(out=ssn[:], in_=sqs[:], axis=AX.X)
            nrm = s2t.tile([P, 1], F32, tag=f"nrm{tag}", name=f"nrm{tag}")
            nc.scalar.activation(nrm[:], ssn[:], AF.Sqrt, scale=TAU)
            rn = s2t.tile([P, 1], F32, tag=f"rn{tag}", name=f"rn{tag}")
            nc.vector.reciprocal(rn[:], nrm[:])
            nc.vector.tensor_scalar_mul(z[:], z[:], rn[:, 0:1])

        def transpose_into(dst, dcol, dstride, z, P):
            """dst[:, k*dstride + dcol ...+P] <- z[P,H] chunk k transposed."""
            for k in range(6):
                pt = psT.tile([128, 128], F32, tag="pt")
                nc.tensor.transpose(
                    pt[:, 0:P], z[:, k * 128 : (k + 1) * 128], idt[0:P, 0:P]
                )
                nc.vector.tensor_copy(
                    dst[:, k * dstride + dcol : k * dstride + dcol + P],
                    pt[:, 0:P],
                )

        def consume_z64():
            zh = s2.tile([N_PAIR, H], F32)
            nc.gpsimd.dma_start(
                zh[:], cc_o.rearrange("(c j) e -> j c e", c=N_CORES)
            )
            normalize(zh, N_PAIR, "z64")
            transpose_into(zT64, 0, N_PAIR, zh, N_PAIR)

        def consume_group(g):
            """Normalize own rows [8g, 8g+8) into zownT columns."""
            src = cc_in if g == 0 else rt[g - 1]
            zg = s2t.tile([8, H], F32, tag="zg", name=f"zg{g}")
            nc.gpsimd.dma_start(zg[:], src[:])
            normalize(zg, 8, "g")
            transpose_into(zownT, 8 * g, B_SH, zg, 8)

        def s_block(g):
            """pS[:, 8g:8g+8] = sum_k zT64_k.T @ zownT_k[:, 8g:8g+8]; exp."""
            for k in range(6):
                nc.tensor.matmul(
                    pS[:, 8 * g : 8 * g + 8],
                    lhsT=zT64[:, k * N_PAIR : (k + 1) * N_PAIR],
                    rhs=zownT[:, k * B_SH + 8 * g : k * B_SH + 8 * g + 8],
                    start=(k == 0),
                    stop=(k == 5),
                )
            eg = s2t.tile([N_PAIR, 8], F32, tag="eg", name=f"eg{g}")
            nc.scalar.activation(eg[:], pS[:, 8 * g : 8 * g + 8], AF.Exp,
                                 scale=1.0, accum_out=esum[:, g : g + 1])
            if g == 0:
                pm = s2t.tile([N_PAIR, 8], F32, tag="pm", name="pm")
                nc.vector.tensor_mul(pm[:], pS[:, 0:8], m64[:])
                nc.vector.reduce_sum(out=pay[:, 1:2], in_=pm[:], axis=AX.X)

        def send_group(g):
            """Stage group-g raw sums to DRAM; g==0 also AllGathers them."""
            dst = cc_in if g == 0 else rt[g - 1]
            nc.scalar.dma_start(
                dst[:],
                pooled_sb[0:1, g * 8 * H : (g + 1) * 8 * H].rearrange(
                    "o (b e) -> o b e", e=H
                ),
            )
            if g == 0:
                if use_collective:
                    nc.gpsimd.collective_compute(
                        "AllGather",
                        mybir.AluOpType.bypass,
                        replica_groups=[list(range(N_CORES))],
                        ins=[cc_in[:].opt()],
                        outs=[cc_o[:].opt()],
                    )
                else:
                    for c in range(N_CORES):
                        nc.scalar.dma_start(cc_o[c * 8 : (c + 1) * 8, :],
                                            cc_in[:])

        # ---- stage 1 + interleaved consume ----------------------------------
        x4 = x.rearrange("b (c p) e -> b p c e", p=128)  # [32, 128, 4, 768]
        for b in range(B_SH):
            if do_s1:
                # float32r: 1 PE cycle/row (fp32 is 4); same 4-byte bits
                xt = xin.tile([128, 4 * H], F32R)
                nc.sync.dma_start(xt[:], x4[b].bitcast(F32R))
                ps = ps1.tile([1, H], F32)
                for c in range(4):
                    nc.tensor.matmul(
                        ps[:, 0:512],
                        lhsT=ones_r[:, 0:1],
                        rhs=xt[:, c * H : c * H + 512],
                        start=(c == 0),
                        stop=(c == 3),
                    )
                for c in range(4):
                    nc.tensor.matmul(
                        ps[:, 512:H],
                        lhsT=ones_r[:, 0:1],
                        rhs=xt[:, c * H + 512 : (c + 1) * H],
                        start=(c == 0),
                        stop=(c == 3),
                    )
                nc.scalar.copy(pooled_sb[0:1, b * H : (b + 1) * H], ps[:])
            if do_cc:
                if b % 8 == 7:
                    send_group(b // 8)
                if do_s2:
                    if b == 15:
                        consume_group(0)
                        consume_z64()
                        s_block(0)
                    elif b == 23:
                        consume_group(1)
                        s_block(1)
                        consume_group(2)
                        s_block(2)
                    elif b == 31:
                        consume_group(3)
                        s_block(3)

        if not do_s2:
            return

        # ---- finish: payload AllGather + logden + pair sum ------------------
        nc.vector.reduce_sum(out=pay[:, 0:1], in_=esum[:], axis=AX.X)
        pT = psS.tile([2, N_PAIR], F32, tag="pT")
        nc.tensor.transpose(pT[:, 0:N_PAIR], pay[:], idt[0:N_PAIR, 0:N_PAIR])
        pTs = s2.tile([2, N_PAIR], F32)
        nc.vector.tensor_copy(pTs[:], pT[:, 0:N_PAIR])
        nc.scalar.dma_start(cc2_in[:], pTs[:])
        if use_collective:
            nc.gpsimd.collective_compute(
                "AllGather",
                mybir.AluOpType.bypass,
                replica_groups=[list(range(N_CORES))],
                ins=[cc2_in[:].opt()],
                outs=[cc2_o[:].opt()],
            )
        else:
            for c in range(N_CORES):
                nc.scalar.dma_start(cc2_o[c * 2 : (c + 1) * 2, :], cc2_in[:])

        # pb[(c,v), i] = payload row v of rank c; rank-sum via two selector
        # matmuls (sel col 0 picks v=0 rows, col 1 picks v=1)
        pb = s2.tile([2 * N_CORES, N_PAIR], F32)
        nc.gpsimd.dma_start(pb[:], cc2_o[:])
        psEP = psS.tile([1, 2 * N_PAIR], F32, tag="psEP")
        nc.tensor.matmul(psEP[:, 0:N_PAIR], lhsT=C["sel"][:, 0:1], rhs=pb[:],
                         start=True, stop=True)
        nc.tensor.matmul(psEP[:, N_PAIR : 2 * N_PAIR], lhsT=C["sel"][:, 1:2],
                         rhs=pb[:], start=True, stop=True)
        den = s2.tile([1, N_PAIR], F32)
        nc.vector.tensor_scalar_add(den[:], psEP[:, 0:N_PAIR], -E2)
        ld = s2.tile([1, N_PAIR], F32)
        nc.scalar.activation(ld[:], den[:], AF.Ln)
        t1 = s2.tile([1, N_PAIR], F32)
        nc.vector.tensor_mul(t1[:], ld[:], C["cntR"][:])
        pr = s2.tile([1, N_PAIR], F32)
        nc.vector.tensor_sub(pr[:], t1[:], psEP[:, N_PAIR : 2 * N_PAIR])
        tot = s2.tile([1, 1], F32)
        nc.vector.reduce_sum(out=tot[:], in_=pr[:], axis=AX.X)
        res = s2.tile([1, 1], F32)
        nc.vector.tensor_scalar_mul(res[:], tot[:], -2.0 / N_PAIR * (N_PAIR - 1))
        nc.gpsimd.dma_start(out[0:1, 0:1], res[:])


def build_nc(reps=1, stages=("s1", "cc", "s2"), use_collective=True):
    nc = bacc.Bacc("TRN2", target_bir_lowering=False, debug=False,
                   num_devices=N_CORES)
    x = nc.dram_tensor("x", [B_SH, S, H], F32, kind="ExternalInput")
    ident = nc.dram_tensor("ident", [128, 128], F32, kind="ExternalInput")
    mask64 = nc.dram_tensor("mask64", [N_PAIR, 8], F32, kind="ExternalInput")
    cnt = nc.dram_tensor("cnt", [N_PAIR, 1], F32, kind="ExternalInput")
    sel = nc.dram_tensor("sel", [2 * N_CORES, 2], F32, kind="ExternalInput")
    out = nc.dram_tensor("loss", [1, 1], F32, kind="ExternalOutput")
    with tile.TileContext(nc) as tc:
        with ExitStack() as ctx:
            C = _consts(tc, ctx, ident.ap(), mask64.ap(), cnt.ap(), sel.ap())
            for _ in range(reps):
                _body(tc, C, x.ap(), out.ap(),
                      use_collective=use_collective, stages=stages)
    nc.compile()
    return nc


def const_inputs(core_id):
    ident = np.eye(128, dtype=np.float32)
    i = np.arange(N_PAIR)[:, None]
    l = (core_id + 8 * np.arange(8))[None, :]
    mask64 = (i < l).astype(np.float32)  # strict upper triangle, own columns
    cnt = (N_PAIR - 1 - np.arange(N_PAIR, dtype=np.float32)).reshape(N_PAIR, 1)
    sel = np.zeros((2 * N_CORES, 2), dtype=np.float32)
    sel[0::2, 0] = 1.0  # rank-sum selector for payload col 0 (Esum)
    sel[1::2, 1] = 1.0  # payload col 1 (pair rows)
    return {"ident": ident, "mask64": mask64, "cnt": cnt, "sel": sel}


def make_in_maps(last_hidden_states, input_mask):
    del input_mask  # cancels exactly in the L2 normalization
    x = np.asarray(last_hidden_states, dtype=np.float32)
    return [
        {"x": np.ascontiguousarray(x[c::N_CORES]), **const_inputs(c)}
        for c in range(N_CORES)
    ]


_CACHE = {}


def get_nc():
    if "nc" not in _CACHE:
        _CACHE["nc"] = build_nc()
    return _CACHE["nc"]


def kernel(last_hidden_states, input_mask):
    nc = get_nc()
    in_maps = make_in_maps(last_hidden_states, input_mask)
    res = bass_utils.run_bass_kernel_spmd(nc, in_maps, core_ids=list(range(N_CORES)))
    return np.asarray(res.results[0]["loss"], dtype=np.float32).reshape(())


# revision 28
# speedup vs baseline: 1.6016x; 1.1747x over previous
"""Trainium2 Bass kernel: BertCL mean-pool + NT-Xent contrastive loss (v2).

Contract: kernel(last_hidden_states [256,512,768] f32, input_mask [256,512] f32)
-> scalar f32 loss, numerically matching the jax reference.

Strategy (8 NeuronCores, SPMD), v2 = distributed logits:
  Batch axis sharded STRIDED: core c owns logical batches {c, c+8, ...}
  (local j <-> logical c + 8j).

  stage 1 (memory-bound): per local batch, stream [512,768] through SBUF as a
    [128, 4*768] tile and reduce the sequence axis with ones-vector matmuls
    (float32r: 1 PE cycle/row vs fp32's 4) accumulating in PSUM -> [1,768]
    raw sums staged into one SBUF row.  (The reference's division by the mask
    row-sum is a positive per-row scalar that cancels exactly in the L2
    normalization, so raw sums suffice.)

  Distributed loss: only logical rows 0..63 (= every core's locals j<8) are
    ever *anchor* rows.  After local batch 7, AllGather those 64 raw rows
    (24KB, fully hidden under streaming); every core normalizes them into
    z64T [e x 64].  Each core normalizes its OWN 32 rows (via a local DRAM
    round-trip per 8-row group) into zownT columns and accumulates the one
    S block it owns: S[0:64, own] = z64 @ z_own^T.  exp+row-accumulate give
    the per-core partial denominator Esum_own[64]; the strict-upper-triangle
    pair term uses only the group-0 columns with a per-core data mask.
    The FINAL collective is an AllGather of a [64,2] payload (512B):
    (Esum_own | pair-row-sums).  Rank-summing uses two selector matmuls,
    logden = ln(Esum-e^2) (diag exp(S_ii)=e^{1/tau} subtracted exactly), then
    loss = -2/n*(n-1) * (sum_i cnt_i*logden_i - pair_total).

  Engine discipline: the SP(sync) ring carries ONLY the 32 x-stream DMAs so
    it never head-blocks on a collective semaphore; DRAM stage-writes on the
    Act(scalar) HWDGE ring; consume loads, collectives, and all post-AG2
    DMAs on gpsimd so the Act ring frees up for the next rep's PSUM drains.

  NOTE: fused DVE ops (tensor_tensor_reduce, scalar_tensor_tensor) pass
  CoreSim but hang/crash this hardware - only plain DVE ops are used.
"""

import sys
from contextlib import ExitStack

import numpy as np

_REPO = "/opt/trn_rl_repo"
if _REPO not in sys.path:
    sys.path.insert(0, _REPO)

import concourse.bass as bass  # noqa: E402  (kept for callers/debugging)
import concourse.tile as tile  # noqa: E402
from concourse import bacc, bass_utils, mybir  # noqa: E402

N_CORES = 8
B, S, H = 256, 512, 768
B_SH = B // N_CORES  # 32 local batches per core
N_PAIR = B // 4  # 64 anchor rows
G = 4  # own-row groups of 8
TAU = 0.5
E2 = float(np.exp(np.float64(1.0 / TAU)))  # exp(S_ii) subtracted from denom
F32 = mybir.dt.float32
F32R = mybir.dt.float32r
AX = mybir.AxisListType
AF = mybir.ActivationFunctionType


def _consts(tc, ctx, ident, mask64, cnt, sel):
    nc = tc.nc
    const = ctx.enter_context(tc.tile_pool(name="const", bufs=1))
    c = {}
    c["ones"] = const.tile([128, 1], F32, name="ones")
    nc.vector.memset(c["ones"][:], 1.0)
    c["ones_r"] = const.tile([128, 1], F32R, name="ones_r")
    nc.vector.tensor_copy(c["ones_r"][:], c["ones"][:])  # memset can't do f32r
    c["idt"] = const.tile([128, 128], F32, name="idt")
    nc.scalar.dma_start(c["idt"][:], ident[:])
    c["m64"] = const.tile([N_PAIR, 8], F32, name="m64")
    nc.scalar.dma_start(c["m64"][:], mask64[:])
    c["cntR"] = const.tile([1, N_PAIR], F32, name="cntR")  # cnt as a row
    nc.scalar.dma_start(c["cntR"][:], cnt.rearrange("i o -> o i"))
    c["sel"] = const.tile([2 * N_CORES, 2], F32, name="selt")
    nc.scalar.dma_start(c["sel"][:], sel[:])
    return c


def _body(tc, C, x, out, use_collective=True, stages=("s1", "cc", "s2")):
    nc = tc.nc
    do_s1 = "s1" in stages
    do_cc = "cc" in stages
    do_s2 = "s2" in stages and do_cc
    ones, ones_r, idt, m64 = C["ones"], C["ones_r"], C["idt"], C["m64"]

    with ExitStack() as ctx:
        # Shared collective outputs must have exactly one writer, so the
        # DRAM tiles are per-rep
        dram = ctx.enter_context(tc.tile_pool(name="dram", bufs=1, space="DRAM"))
        cc_in = dram.tile([8, H], F32, name="cc_in")
        cc_o = dram.tile([64, H], F32, addr_space="Shared", name="cc_o")
        cc2_in = dram.tile([2, N_PAIR], F32, name="cc2_in")
        cc2_o = dram.tile([2 * N_CORES, N_PAIR], F32, addr_space="Shared",
                          name="cc2_o")

        work = ctx.enter_context(tc.tile_pool(name="work", bufs=1))
        pooled_sb = work.tile([1, B_SH * H], F32)

        xin = ctx.enter_context(tc.tile_pool(name="xin", bufs=7))
        ps1 = ctx.enter_context(tc.tile_pool(name="ps1", bufs=2, space="PSUM"))
        s2 = ctx.enter_context(tc.tile_pool(name="s2", bufs=1))
        s2t = ctx.enter_context(tc.tile_pool(name="s2t", bufs=2))
        psT = ctx.enter_context(tc.tile_pool(name="psT", bufs=1, space="PSUM"))
        psS = ctx.enter_context(tc.tile_pool(name="psS", bufs=1, space="PSUM"))

        # zT64[:, k*64 + l] = z64[l, k*128 + q] at partition q
        zT64 = s2.tile([128, 6 * N_PAIR], F32)
        zownT = s2.tile([128, 6 * B_SH], F32)
        pS = psS.tile([N_PAIR, B_SH], F32)
        esum = s2.tile([N_PAIR, G], F32)
        pay = s2.tile([N_PAIR, 2], F32)

        def normalize(z, P, tag):
            """L2-normalize rows of z [P, H] in place, folding 1/sqrt(tau)."""
            sqs = s2t.tile([P, H], F32, tag=f"sqs{tag}", name=f"sqs{tag}")
            ssn = s2t.tile([P, 1], F32, tag=f"ssn{tag}", name=f"ssn{tag}")
            nc.vector.tensor_mul(sqs[:], z[:], z[:])
            nc.vector.reduce_sum(out=ssn[:], in_=sqs[:], axis=AX.X)
            nrm = s2t.tile([P, 1], F32, tag=f"nrm{tag}", name=f"nrm{tag}")
            nc.scalar.activation(nrm[:], ssn[:], AF.Sqrt, scale=TAU)
            rn = s2t.tile([P, 1], F32, tag=f"rn{tag}", name=f"rn{tag}")
            nc.vector.reciprocal(rn[:], nrm[:])
            nc.vector.tensor_scalar_mul(z[:], z[:], rn[:, 0:1])

        def transpose_into(dst, dcol, dstride, z, P):
            """dst[:, k*dstride + dcol ...+P] <- z[P,H] chunk k transposed."""
            for k in range(6):
                pt = psT.tile([128, 128], F32, tag="pt")
                nc.tensor.transpose(
                    pt[:, 0:P], z[:, k * 128 : (k + 1) * 128], idt[0:P, 0:P]
                )
                nc.vector.tensor_copy(
                    dst[:, k * dstride + dcol : k * dstride + dcol + P],
                    pt[:, 0:P],
                )

        def consume_z64():
            zh = s2.tile([N_PAIR, H], F32)
            nc.gpsimd.dma_start(
                zh[:], cc_o.rearrange("(c j) e -> j c e", c=N_CORES)
            )
            normalize(zh, N_PAIR, "z64")
            transpose_into(zT64, 0, N_PAIR, zh, N_PAIR)

        def consume_group(g):
            """Normalize own rows [8g, 8g+8) into zownT columns.

            SBUF->SBUF partition-scatter straight from the pooled row (no
            DRAM round-trip; SBUF transfers have no small-descriptor
            penalty)."""
            zg = s2t.tile([8, H], F32, tag="zg", name=f"zg{g}")
            nc.gpsimd.dma_start(
                zg[:],
                pooled_sb[0:1, g * 8 * H : (g + 1) * 8 * H].rearrange(
                    "o (b e) -> o b e", e=H
                ),
            )
            normalize(zg, 8, "g")
            transpose_into(zownT, 8 * g, B_SH, zg, 8)

        def s_block(g):
            """pS[:, 8g:8g+8] = sum_k zT64_k.T @ zownT_k[:, 8g:8g+8]; exp."""
            for k in range(6):
                nc.tensor.matmul(
                    pS[:, 8 * g : 8 * g + 8],
                    lhsT=zT64[:, k * N_PAIR : (k + 1) * N_PAIR],
                    rhs=zownT[:, k * B_SH + 8 * g : k * B_SH + 8 * g + 8],
                    start=(k == 0),
                    stop=(k == 5),
                )
            eg = s2t.tile([N_PAIR, 8], F32, tag="eg", name=f"eg{g}")
            nc.scalar.activation(eg[:], pS[:, 8 * g : 8 * g + 8], AF.Exp,
                                 scale=1.0, accum_out=esum[:, g : g + 1])
            if g == 0:
                pm = s2t.tile([N_PAIR, 8], F32, tag="pm", name="pm")
                nc.vector.tensor_mul(pm[:], pS[:, 0:8], m64[:])
                nc.vector.reduce_sum(out=pay[:, 1:2], in_=pm[:], axis=AX.X)

        def send_group(g):
            """g==0: stage group-0 raw sums to DRAM and AllGather them."""
            if g != 0:
                return
            nc.scalar.dma_start(
                cc_in[:],
                pooled_sb[0:1, 0 : 8 * H].rearrange("o (b e) -> o b e", e=H),
            )
            if use_collective:
                nc.gpsimd.collective_compute(
                    "AllGather",
                    mybir.AluOpType.bypass,
                    replica_groups=[list(range(N_CORES))],
                    ins=[cc_in[:].opt()],
                    outs=[cc_o[:].opt()],
                )
            else:
                for c in range(N_CORES):
                    nc.scalar.dma_start(cc_o[c * 8 : (c + 1) * 8, :],
                                        cc_in[:])

        # ---- stage 1 + interleaved consume ----------------------------------
        x4 = x.rearrange("b (c p) e -> b p c e", p=128)  # [32, 128, 4, 768]
        for b in range(B_SH):
            if do_s1:
                # float32r: 1 PE cycle/row (fp32 is 4); same 4-byte bits
                xt = xin.tile([128, 4 * H], F32R)
                nc.sync.dma_start(xt[:], x4[b].bitcast(F32R))
                ps = ps1.tile([1, H], F32)
                for c in range(4):
                    nc.tensor.matmul(
                        ps[:, 0:512],
                        lhsT=ones_r[:, 0:1],
                        rhs=xt[:, c * H : c * H + 512],
                        start=(c == 0),
                        stop=(c == 3),
                    )
                for c in range(4):
                    nc.tensor.matmul(
                        ps[:, 512:H],
                        lhsT=ones_r[:, 0:1],
                        rhs=xt[:, c * H + 512 : (c + 1) * H],
                        start=(c == 0),
                        stop=(c == 3),
                    )
                nc.scalar.copy(pooled_sb[0:1, b * H : (b + 1) * H], ps[:])
            if do_cc:
                if b % 8 == 7:
                    send_group(b // 8)
                if do_s2:
                    if b == 15:
                        consume_group(0)
                        consume_z64()
                        s_block(0)
                    elif b == 23:
                        consume_group(1)
                        s_block(1)
                        consume_group(2)
                        s_block(2)
                    elif b == 31:
                        consume_group(3)
                        s_block(3)

        if not do_s2:
            return

        # ---- finish: payload AllGather + logden + pair sum ------------------
        nc.vector.reduce_sum(out=pay[:, 0:1], in_=esum[:], axis=AX.X)
        pT = psS.tile([2, N_PAIR], F32, tag="pT")
        nc.tensor.transpose(pT[:, 0:N_PAIR], pay[:], idt[0:N_PAIR, 0:N_PAIR])
        pTs = s2.tile([2, N_PAIR], F32)
        nc.vector.tensor_copy(pTs[:], pT[:, 0:N_PAIR])
        nc.scalar.dma_start(cc2_in[:], pTs[:])
        if use_collective:
            nc.gpsimd.collective_compute(
                "AllGather",
                mybir.AluOpType.bypass,
                replica_groups=[list(range(N_CORES))],
                ins=[cc2_in[:].opt()],
                outs=[cc2_o[:].opt()],
            )
        else:
            for c in range(N_CORES):
                nc.scalar.dma_start(cc2_o[c * 2 : (c + 1) * 2, :], cc2_in[:])

        # pb[(c,v), i] = payload row v of rank c; rank-sum via two selector
        # matmuls (sel col 0 picks v=0 rows, col 1 picks v=1)
        pb = s2.tile([2 * N_CORES, N_PAIR], F32)
        nc.gpsimd.dma_start(pb[:], cc2_o[:])
        psEP = psS.tile([1, 2 * N_PAIR], F32, tag="psEP")
        nc.tensor.matmul(psEP[:, 0:N_PAIR], lhsT=C["sel"][:, 0:1], rhs=pb[:],
                         start=True, stop=True)
        nc.tensor.matmul(psEP[:, N_PAIR : 2 * N_PAIR], lhsT=C["sel"][:, 1:2],
                         rhs=pb[:], start=True, stop=True)
        den = s2.tile([1, N_PAIR], F32)
        nc.vector.tensor_scalar_add(den[:], psEP[:, 0:N_PAIR], -E2)
        ld = s2.tile([1, N_PAIR], F32)
        nc.scalar.activation(ld[:], den[:], AF.Ln)
        t1 = s2.tile([1, N_PAIR], F32)
        nc.vector.tensor_mul(t1[:], ld[:], C["cntR"][:])
        pr = s2.tile([1, N_PAIR], F32)
        nc.vector.tensor_sub(pr[:], t1[:], psEP[:, N_PAIR : 2 * N_PAIR])
        tot = s2.tile([1, 1], F32)
        nc.vector.reduce_sum(out=tot[:], in_=pr[:], axis=AX.X)
        res = s2.tile([1, 1], F32)
        nc.vector.tensor_scalar_mul(res[:], tot[:], -2.0 / N_PAIR * (N_PAIR - 1))
        nc.gpsimd.dma_start(out[0:1, 0:1], res[:])


def build_nc(reps=1, stages=("s1", "cc", "s2"), use_collective=True):
    nc = bacc.Bacc("TRN2", target_bir_lowering=False, debug=False,
                   num_devices=N_CORES)
    x = nc.dram_tensor("x", [B_SH, S, H], F32, kind="ExternalInput")
    ident = nc.dram_tensor("ident", [128, 128], F32, kind="ExternalInput")
    mask64 = nc.dram_tensor("mask64", [N_PAIR, 8], F32, kind="ExternalInput")
    cnt = nc.dram_tensor("cnt", [N_PAIR, 1], F32, kind="ExternalInput")
    sel = nc.dram_tensor("sel", [2 * N_CORES, 2], F32, kind="ExternalInput")
    out = nc.dram_tensor("loss", [1, 1], F32, kind="ExternalOutput")
    with tile.TileContext(nc) as tc:
        with ExitStack() as ctx:
            C = _consts(tc, ctx, ident.ap(), mask64.ap(), cnt.ap(), sel.ap())
            for _ in range(reps):
                _body(tc, C, x.ap(), out.ap(),
                      use_collective=use_collective, stages=stages)
    nc.compile()
    return nc


def const_inputs(core_id):
    ident = np.eye(128, dtype=np.float32)
    i = np.arange(N_PAIR)[:, None]
    l = (core_id + 8 * np.arange(8))[None, :]
    mask64 = (i < l).astype(np.float32)  # strict upper triangle, own columns
    cnt = (N_PAIR - 1 - np.arange(N_PAIR, dtype=np.float32)).reshape(N_PAIR, 1)
    sel = np.zeros((2 * N_CORES, 2), dtype=np.float32)
    sel[0::2, 0] = 1.0  # rank-sum selector for payload col 0 (Esum)
    sel[1::2, 1] = 1.0  # payload col 1 (pair rows)
    return {"ident": ident, "mask64": mask64, "cnt": cnt, "sel": sel}


def make_in_maps(last_hidden_states, input_mask):
    del input_mask  # cancels exactly in the L2 normalization
    x = np.asarray(last_hidden_states, dtype=np.float32)
    return [
        {"x": np.ascontiguousarray(x[c::N_CORES]), **const_inputs(c)}
        for c in range(N_CORES)
    ]


_CACHE = {}


def get_nc():
    if "nc" not in _CACHE:
        _CACHE["nc"] = build_nc()
    return _CACHE["nc"]


def kernel(last_hidden_states, input_mask):
    nc = get_nc()
    in_maps = make_in_maps(last_hidden_states, input_mask)
    res = bass_utils.run_bass_kernel_spmd(nc, in_maps, core_ids=list(range(N_CORES)))
    return np.asarray(res.results[0]["loss"], dtype=np.float32).reshape(())
